# revision 22
# baseline (speedup 1.0000x reference)
"""GNN message-passing discriminator on 8 trn2 NeuronCores.

Strategy (edge-parallel by *destination* node, DRAM-resident node features):
  - Nodes sharded npc/core; each edge lives on the core owning its dst.
  - Per-layer node features live in a DRAM table of 256B rows in chunk-major
    slot order (chunk of CHUNK_W windows, then rank, then node). As each
    7-window chunk of updated nodes finishes, the core casts it to fp8
    (DVE copy, off the Pool sequencer), stages it node-major with an SP DMA,
    and AllGathers the chunk -- so the collectives pipeline behind compute
    instead of serializing at the layer boundary. At end of layer, cheap
    SWDGE cast-DMAs expand the fp8 chunks into the f16 256B-row table, and
    the next layer gathers src features straight from HBM with a transposed
    dma_gather. No SBUF feature table, no full-size AllGather.
  - Message MLP: TensorE matmul per 128-edge tile; LeakyReLU and the mean's
    1/deg(dst) scale fuse into one ScalarE/DVE activation per tile. Layers
    0/1 emit messages in fp8 and aggregate stream-adjacent tile pairs with
    one DoubleRow matmul (256-edge contraction); layer 2 keeps f16 messages
    for accuracy.
  - Aggregation: per-tile one-hot fp8 selector matrices (host-built,
    streamed from HBM) matmul'd against messages, accumulating
    feature-major per-128-node-window sums in PSUM.
  - Update MLP per window writes node-major into the hn staging buffer that
    feeds the chunked AllGathers.
  - Global mean-pool via per-window batch-selector matmuls; final MLP runs
    replicated on every core.

Host-side work is integer index prep (sort/bincount/one-hot selectors) and
layout/dtype staging; all float compute runs on device.
"""

import numpy as np

import concourse.bass as bass
import concourse.bacc as bacc
import concourse.mybir as mybir
import concourse.tile as tile
from concourse.bass_utils import run_bass_kernel_spmd

F32 = mybir.dt.float32
F16 = mybir.dt.float16
F8 = mybir.dt.float8e4
I16 = mybir.dt.int16
I32 = mybir.dt.int32
AF = mybir.ActivationFunctionType
NP_F8 = mybir.dt.np(F8)

N_GRAPHS = 32
HID = [64, 128, 256]
MLP_DIMS = [256, 128, 64, 1]
N_CORES = 8

ELEM = 128      # fp16 feature slots per table row upper half (256 bytes)
ROW = 256       # full table row in fp16 elems (512 bytes, KV-page style)
WIN = 128       # nodes per aggregation window
GROUP_W = 7     # windows per gather group
CHUNK_W = 7     # windows per AllGather-input staging DMA
LRELU = 0.2


def _cdiv(a, b):
    return -(-a // b)


_LRELU_OP = None


def _get_lrelu_op():
    """out = max(s*x, 0.2*s*x) in one DVE pass (s per-partition, 0.2 imm)."""
    global _LRELU_OP
    if _LRELU_OP is not None:
        return _LRELU_OP
    import concourse.dve_ops as dops
    from concourse.dve_spec import Spec, Src0, C0, C2, maxx
    name = "LRELU_SCALE_ANT"
    if name not in dops._SUB_OPCODE_FOR_NAME:
        row = max(dops._SUB_OPCODE_FOR_NAME.values()) + 1
        assert row < 0x20
        dops._SUB_OPCODE_FOR_NAME[name] = row
    spec = Spec(
        body=maxx(Src0 * C0, Src0 * C0 * C2),
        reference=lambda in0, in1, c0, c1, c2: np.maximum(
            in0 * c0, in0 * c0 * c2),
    )
    shas = {}
    for ver in ("v3", "v4"):
        try:
            probe = dops.DveOp(name, spec, subdim=False, uops_sha={})
            probe.compile(ver)
        except ValueError as ex:
            import re
            shas[ver] = re.search(r"\(" + ver + r": ([0-9a-f]+)", str(ex)).group(1)
    op = dops.DveOp(name, spec, subdim=False, uops_sha=shas)
    if not any(o.name == name for o in dops.OPS):
        dops.OPS.append(op)
    dops.CUSTOM_DVE_SPECS[name] = spec
    _LRELU_OP = op
    return op


class Cfg:
    pass


# ============================================================ host index prep
def host_prep(inputs, n_cores=N_CORES):
    x = np.asarray(inputs["x"], np.float32)
    ei = np.asarray(inputs["edge_index"], np.int64)
    ea = np.asarray(inputs["edge_attr"], np.float32)
    batch = np.asarray(inputs["batch"], np.int64)

    n_nodes, node_dim = x.shape
    n_edges = ei.shape[1]

    cfg = Cfg()
    cfg.n_cores = n_cores
    cfg.n_nodes = n_nodes
    cfg.node_dim = node_dim
    cfg.n_graphs = N_GRAPHS
    npc = n_nodes // n_cores
    assert npc * n_cores == n_nodes
    cfg.npc = npc
    cfg.n_win = _cdiv(npc, WIN)
    cfg.last_win_nodes = npc - (cfg.n_win - 1) * WIN
    win_pad = cfg.n_win * WIN
    cfg.win_pad = win_pad
    # chunk-major slot space (chunk k of CHUNK_W windows, then rank, then
    # node) so chunked AllGather outputs are contiguous table ranges; gather
    # idx is int16 so bucket-split at 32768
    chunks = [list(range(g, min(g + CHUNK_W, cfg.n_win)))
              for g in range(0, cfg.n_win, CHUNK_W)]
    cfg.chunks = chunks
    cfg.chunk_rows = [len(ws) * WIN for ws in chunks]
    gb = np.cumsum([0] + [n_cores * cr for cr in cfg.chunk_rows])
    cfg.chunk_gbase = gb[:-1]
    cfg.slots = int(gb[-1])
    assert cfg.slots == n_cores * win_pad
    cfg.buck0 = min(32768, cfg.slots)
    assert cfg.slots - cfg.buck0 < 32768

    src = ei[0].astype(np.int64)
    dst = ei[1].astype(np.int64)
    deg = np.bincount(dst, minlength=n_nodes).astype(np.float32)
    inv_deg = (1.0 / np.maximum(deg, 1.0)).astype(np.float32)

    c_of = np.arange(n_nodes) // npc
    r_of = np.arange(n_nodes) % npc
    w_of = r_of // WIN
    k_of = w_of // CHUNK_W
    cr = np.asarray(cfg.chunk_rows, np.int64)
    gbase = np.asarray(cfg.chunk_gbase, np.int64)
    first_w = np.asarray([ws[0] for ws in chunks], np.int64)
    slot = (gbase[k_of] + c_of * cr[k_of]
            + (r_of - first_w[k_of] * WIN)).astype(np.int64)
    slot_of_node = slot

    sslot = slot[src]
    ecore = dst // npc
    ewin = (dst % npc) // WIN
    ebuck = (sslot >= cfg.buck0).astype(np.int64)

    key = (ecore * cfg.n_win + ewin) * 2 + ebuck
    cnt = np.bincount(key, minlength=n_cores * cfg.n_win * 2).reshape(
        n_cores, cfg.n_win, 2)
    T = np.maximum(_cdiv(cnt.max(axis=0), 128), 1)   # [n_win, 2]
    cfg.T = T
    cfg.n_tiles = int(T.sum())
    cfg.e_pad = cfg.n_tiles * 128

    groups = [list(range(g, min(g + GROUP_W, cfg.n_win)))
              for g in range(0, cfg.n_win, GROUP_W)]
    cfg.groups = groups

    # padded stream order: per group: [A segs of its windows] [B segs]
    seg_off = {}
    pos = 0
    for ws in groups:
        for b in (0, 1):
            for w in ws:
                seg_off[(w, b)] = pos
                pos += int(T[w, b]) * 128
    assert pos == cfg.e_pad
    cfg.seg_off = seg_off

    order = np.lexsort((ebuck, ewin, ecore))
    src_s = sslot[order]
    dst_s = dst[order]
    ea_s = ea[order]
    inv_s = inv_deg[dst[order]]

    ck = (ecore[order] * cfg.n_win + ewin[order]) * 2 + ebuck[order]
    seg_starts = np.searchsorted(ck, np.arange(n_cores * cfg.n_win * 2))
    seg_ends = np.append(seg_starts[1:], n_edges)

    e_pad = cfg.e_pad
    in_maps = []
    wts = _pack_weights(inputs, node_dim)
    ident = np.eye(128, dtype=np.float16)

    for c in range(n_cores):
        g_idx = np.zeros(e_pad, np.int64)
        buck_flag = np.zeros(e_pad, np.bool_)
        e_a = np.zeros((4, e_pad), np.float32)
        invd = np.zeros(e_pad, np.float32)
        selcol = np.full(e_pad, -1, np.int64)

        for w in range(cfg.n_win):
            for b in (0, 1):
                s0 = seg_starts[(c * cfg.n_win + w) * 2 + b]
                s1 = seg_ends[(c * cfg.n_win + w) * 2 + b]
                n = s1 - s0
                o = seg_off[(w, b)]
                assert n <= T[w, b] * 128
                if n:
                    buck_flag[o:o + n] = bool(b)
                    g_idx[o:o + n] = src_s[s0:s1] - (cfg.buck0 if b else 0)
                    e_a[:3, o:o + n] = ea_s[s0:s1].T
                    e_a[3, o:o + n] = 1.0
                    invd[o:o + n] = inv_s[s0:s1]
                    selcol[o:o + n] = (dst_s[s0:s1] % npc) - w * WIN

        gi = np.zeros((128, e_pad // 16), np.int16)
        base = g_idx.astype(np.int16).reshape(-1, 16).T
        for k in range(8):
            gi[16 * k:16 * k + 16] = base

        # layer-0 edge stream: [x[src](10) | ea(3) | 1] fp16, feature-major
        xe = np.zeros((node_dim + 4, e_pad), np.float16)
        edge_valid = selcol >= 0

        sel = np.zeros((128, cfg.n_tiles * 128), np.uint8)
        tt = np.arange(e_pad) // 128
        ee = np.arange(e_pad) % 128
        m = selcol >= 0
        sel[ee[m], tt[m] * 128 + selcol[m]] = 0x38

        xt = np.zeros((node_dim, win_pad), np.float16)
        xt[:, :npc] = x[c * npc:(c + 1) * npc].astype(np.float16).T

        sb = np.zeros((128, cfg.n_win * N_GRAPHS), np.uint8)
        bl = batch[c * npc:(c + 1) * npc].astype(np.int64)
        pp = np.arange(npc) % WIN
        ww = np.arange(npc) // WIN
        sb[pp, ww * N_GRAPHS + bl] = 0x38

        xsrc_slot = np.zeros((cfg.slots, node_dim), np.float16)
        xsrc_slot[slot_of_node] = x.astype(np.float16)
        gsl = g_idx + np.where(buck_flag, cfg.buck0, 0)
        xe[:node_dim, :] = xsrc_slot[gsl].T
        xe[node_dim:node_dim + 4, :] = e_a.astype(np.float16)
        xe[:, ~edge_valid] = 0.0

        m_ = {
            "xeT": xe,
            "gidx": gi,
            "eaT": e_a.astype(np.float16),
            "invd": invd.reshape(-1, 128).T.astype(np.float32).copy(),
            "sel": sel.view(NP_F8),
            "xT_loc": xt,
            "selB": sb.view(NP_F8),
            "ident": ident,
        }
        m_.update(wts)
        in_maps.append(m_)
    return cfg, in_maps


def _pack_weights(inputs, node_dim):
    wts = {}
    node_in = node_dim
    for li in range(len(HID)):
        mw = np.asarray(inputs[f"mw{li}"], np.float32)
        mb = np.asarray(inputs[f"mb{li}"], np.float32)
        wts[f"mwp{li}"] = np.concatenate([mw, mb[None, :]], axis=0)
        wts[f"uw{li}"] = np.asarray(inputs[f"uw{li}"], np.float32)
        wts[f"ub{li}"] = np.asarray(inputs[f"ub{li}"], np.float32)[None, :]
        node_in = HID[li]
    for li in range(len(MLP_DIMS) - 1):
        wts[f"fw{li}"] = np.asarray(inputs[f"fw{li}"], np.float32)
        wts[f"fb{li}"] = np.asarray(
            inputs[f"fb{li}"], np.float32).reshape(-1, 1)
    return wts


# =============================================================== bass builder
def build_program(cfg):
    nc = bacc.Bacc(
        "TRN2",
        target_bir_lowering=False,
        debug=False,
        enable_asserts=False,
        num_devices=cfg.n_cores,
    )
    n_win, n_tiles, e_pad = cfg.n_win, cfg.n_tiles, cfg.e_pad
    win_pad = cfg.win_pad
    NG = cfg.n_graphs
    T = cfg.T
    groups = cfg.groups
    chunks = cfg.chunks
    chunk_rows = cfg.chunk_rows
    chunk_gbase = [int(v) for v in cfg.chunk_gbase]
    core_ids = list(range(cfg.n_cores))
    slots = cfg.slots
    buck0 = cfg.buck0
    n_cores = cfg.n_cores

    D = {}

    def din(name, shape, dt):
        D[name] = nc.dram_tensor(name, list(shape), dt, kind="ExternalInput")

    din("xeT", (cfg.node_dim + 4, e_pad), F16)
    din("gidx", (128, e_pad // 16), I16)
    din("eaT", (4, e_pad), F16)
    din("invd", (128, n_tiles), F32)
    din("sel", (128, n_tiles * 128), F8)
    din("xT_loc", (cfg.node_dim, win_pad), F16)
    din("selB", (128, n_win * NG), F8)
    din("ident", (128, 128), F16)
    node_in = cfg.node_dim
    for li, dout in enumerate(HID):
        din(f"mwp{li}", (node_in + 4, dout), F32)
        din(f"uw{li}", (dout + node_in, dout), F32)
        din(f"ub{li}", (1, dout), F32)
        node_in = dout
    for li in range(len(MLP_DIMS) - 1):
        din(f"fw{li}", (MLP_DIMS[li], MLP_DIMS[li + 1]), F32)
        din(f"fb{li}", (MLP_DIMS[li + 1], 1), F32)
    out_t = nc.dram_tensor("out", [NG, 1], F32, kind="ExternalOutput")

    # group extents in the padded stream
    g_meta = []
    for ws in groups:
        nA = int(sum(T[w, 0] for w in ws)) * 128
        nB = int(sum(T[w, 1] for w in ws)) * 128
        g_meta.append((cfg.seg_off[(ws[0], 0)], nA, nB))
    max_g_cols = max(nA + nB for _, nA, nB in g_meta)
    max_w_cols = int((T[:, 0] + T[:, 1]).max()) * 128

    from contextlib import ExitStack
    with ExitStack() as _es:
        tc = _es.enter_context(tile.TileContext(nc))
        p_res = _es.enter_context(tc.tile_pool(name="res", bufs=1))
        p_wts = _es.enter_context(tc.tile_pool(name="wts", bufs=1))
        p_gath = _es.enter_context(tc.tile_pool(name="gath", bufs=2))
        p_sel = _es.enter_context(tc.tile_pool(name="selp", bufs=2))
        p_gix = _es.enter_context(tc.tile_pool(name="gix", bufs=2))
        p_ivd = _es.enter_context(tc.tile_pool(name="ivd", bufs=2))
        p_ea = _es.enter_context(tc.tile_pool(name="eal2", bufs=2))
        p_msg = _es.enter_context(tc.tile_pool(name="msg", bufs=6))
        p_aggs = _es.enter_context(tc.tile_pool(name="aggs", bufs=4))
        p_hloc = _es.enter_context(tc.tile_pool(name="hloc", bufs=2))
        p_hna = _es.enter_context(tc.tile_pool(name="hna", bufs=2))
        p_hn = _es.enter_context(tc.tile_pool(name="hnext", bufs=3))
        p_small = _es.enter_context(tc.tile_pool(name="small", bufs=1))
        pp_msg = _es.enter_context(tc.tile_pool(name="pmsg", bufs=3, space="PSUM"))
        pp_agg = _es.enter_context(tc.tile_pool(name="pagg", bufs=2, space="PSUM"))
        pp_upd = _es.enter_context(tc.tile_pool(name="pupd", bufs=2, space="PSUM"))
        pp_pool = _es.enter_context(tc.tile_pool(name="ppool", bufs=1, space="PSUM"))
        p_dram = _es.enter_context(tc.tile_pool(name="dram", bufs=1, space="DRAM"))
        if True:
            selB_sb = p_res.tile([128, n_win * NG], F8, tag="selB")
            ident_sb = p_res.tile([128, 128], F16, tag="ident")
            ones_row = p_res.tile([1, 128], F16, tag="ones_r")
            ones_col = p_res.tile([128, 1], F16, tag="ones_c")

            nc.sync.dma_start(selB_sb[:], D["selB"][:])
            nc.sync.dma_start(ident_sb[:], D["ident"][:])
            nc.vector.memset(ones_row[:], 1.0)
            nc.vector.memset(ones_col[:], 1.0)

            # weights -> SBUF fp16 (cast during SWDGE DMA)
            W = {}
            node_in = cfg.node_dim
            for li, dout in enumerate(HID):
                mw_chunks = []
                for k, r in enumerate(range(0, node_in + 4, 128)):
                    r1 = min(r + 128, node_in + 4)
                    t = p_wts.tile([r1 - r, dout], F16, tag=f"mwp{li}_{k}")
                    nc.gpsimd.dma_start(t[:], D[f"mwp{li}"][r:r1, :])
                    mw_chunks.append(t)
                W[f"mwp{li}"] = mw_chunks
                chunks_w = []
                for r in list(range(0, dout, 128)):
                    chunks_w.append((r, min(r + 128, dout)))
                for r in list(range(0, node_in, 128)):
                    chunks_w.append((dout + r, dout + min(r + 128, node_in)))
                uws = []
                for k, (r0, r1) in enumerate(chunks_w):
                    t = p_wts.tile([r1 - r0, dout], F16, tag=f"uw{li}_{k}")
                    nc.gpsimd.dma_start(t[:], D[f"uw{li}"][r0:r1, :])
                    uws.append(t)
                W[f"uw{li}"] = uws
                t = p_wts.tile([1, dout], F16, tag=f"ub{li}")
                nc.gpsimd.dma_start(t[:], D[f"ub{li}"][:])
                W[f"ub{li}"] = t
                node_in = dout
            for li in range(len(MLP_DIMS) - 1):
                fws = []
                for k, r in enumerate(range(0, MLP_DIMS[li], 128)):
                    r1 = min(r + 128, MLP_DIMS[li])
                    t = p_wts.tile([r1 - r, MLP_DIMS[li + 1]], F16,
                                   tag=f"fw{li}_{k}")
                    nc.gpsimd.dma_start(t[:], D[f"fw{li}"][r:r1, :])
                    fws.append(t)
                W[f"fw{li}"] = fws
                t = p_wts.tile([MLP_DIMS[li + 1], 1], F32, tag=f"fb{li}")
                nc.sync.dma_start(t[:], D[f"fb{li}"][:])
                W[f"fb{li}"] = t

            hlocT = p_hloc.tile([cfg.node_dim, win_pad], F16, tag="hloc")
            nc.sync.dma_start(hlocT[:], D["xT_loc"][:])

            # per-layer DRAM feature tables (256B rows for dma_gather).
            # L0's h1 (64 feats) AllGathers tight in fp8, then a local
            # cast-DMA expands each chunk into H0; L1's h2 (128 feats)
            # AllGathers straight into H1.
            ag_in = {}
            ag_out = {}
            H = {}
            for li, hd in ((0, 64), (1, 128)):
                ag_in[li] = p_dram.tile([win_pad, hd], F8, tag=f"agi{li}",
                                        name=f"agi{li}")
                ag_out[li] = p_dram.tile([slots, hd], F8, tag=f"ago{li}",
                                         name=f"ago{li}")
                H[li] = p_dram.tile([slots, ELEM], F16, tag=f"H{li}",
                                    name=f"H{li}")

            FP = HID[2] + 1
            gp_in = p_dram.tile([NG, FP], F32, tag="gpi")
            gp_out = p_dram.tile([cfg.n_cores * NG, FP], F32, tag="gpo",
                                 addr_space="Shared")
            psum_pool = pp_pool.tile([NG, FP], F32)

            # ========================= layers
            node_in = cfg.node_dim
            gixall = None
            for li, dout in enumerate(HID):
                mwp = W[f"mwp{li}"]
                nch = _cdiv(dout, 128)
                last = li == len(HID) - 1
                hnextT = None
                hna = None
                if not last:
                    hnextT = p_hloc.tile([dout, win_pad], F16, tag="hloc")
                    hna = p_hna.tile([128, win_pad], F16, tag="hna")
                    if dout < 128:
                        # pad feature columns feed H's v-rows (gathered but
                        # never consumed); zero them so sims see no uninit
                        nc.vector.memset(hna[:], 0.0)

                for gi_, ws in enumerate(groups):
                    start, nA, nB = g_meta[gi_]
                    ncols = nA + nB
                    selbuf = p_sel.tile([128, max_g_cols], F8, tag="selp")
                    ivdb = p_ivd.tile([128, max_g_cols // 128], F32, tag="ivd")
                    nc.sync.dma_start(
                        ivdb[:, 0:ncols // 128],
                        D["invd"][:, start // 128:(start + ncols) // 128])
                    nc.sync.dma_start(
                        selbuf[:, 0:ncols],
                        D["sel"][:, start:start + ncols])
                    if li == 0:
                        gbuf = p_gath.tile([cfg.node_dim + 4, max_g_cols],
                                           F16, tag="gath", name="gbuf")
                        nc.sync.dma_start(
                            gbuf[:, 0:ncols],
                            D["xeT"][:, start:start + ncols])
                    else:
                        gbuf = p_gath.tile([128, max_g_cols], F16, tag="gath",
                                           name="gbuf")
                        Hsrc = H[li - 1]
                        for b, coff, n_b in ((0, 0, nA), (1, nA, nB)):
                            if n_b == 0:
                                continue
                            r0 = 0 if b == 0 else buck0
                            r1 = buck0 if b == 0 else slots
                            if r1 <= r0:
                                # bucket empty (all-pad tiles): any valid rows
                                r0, r1 = 0, slots
                            nc.gpsimd.dma_gather(
                                gbuf[:, coff:coff + n_b].rearrange(
                                    "p (o n) -> p o n", o=1),
                                Hsrc[r0:r1, :],
                                gixall[:, (start + coff) // 16:
                                       (start + coff + n_b) // 16],
                                num_idxs=n_b,
                                num_idxs_reg=n_b,
                                elem_size=ELEM,
                                transpose=True,
                                single_packet=False,
                            )
                        if node_in + 4 <= 128:
                            nc.sync.dma_start(
                                gbuf[node_in:node_in + 4, 0:ncols],
                                D["eaT"][:, start:start + ncols])

                    # window-major processing; stream offsets are bucket-major
                    aoff = 0
                    boff = nA
                    for w in ws:
                        t0, t1 = int(T[w, 0]), int(T[w, 1])
                        wcols = []
                        for k in range(t0):
                            wcols.append(aoff + k * 128)
                        for k in range(t1):
                            wcols.append(boff + k * 128)
                        aoff += t0 * 128
                        boff += t1 * 128

                        eabuf = None
                        if node_in + 4 > 128:
                            eabuf = p_ea.tile([4, max_w_cols], F16, tag="eal2")
                            nc.sync.dma_start(
                                eabuf[:, 0:t0 * 128],
                                D["eaT"][:, start + wcols[0]:
                                         start + wcols[0] + t0 * 128])
                            nc.sync.dma_start(
                                eabuf[:, t0 * 128:(t0 + t1) * 128],
                                D["eaT"][:, start + wcols[t0]:
                                         start + wcols[t0] + t1 * 128])

                        pagg = pp_agg.tile([128, nch * 128], F32, tag="pagg")
                        # L0/L1: messages come out in fp8 and stream-adjacent
                        # tile pairs aggregate with one DoubleRow matmul
                        # (256-edge contraction). L2 keeps fp16 messages for
                        # accuracy (it feeds h3 / the output most directly).
                        pair_fp8 = li < 2
                        msg_dt = F8 if pair_fp8 else F16
                        sched = []
                        for rs, re_ in ((0, t0), (t0, t0 + t1)):
                            k = rs
                            while pair_fp8 and k + 1 < re_:
                                sched.append((k, k + 1))
                                k += 2
                            while k < re_:
                                sched.append((k,))
                                k += 1
                        n_em = len(sched)

                        def _msg_one(k, col, out_ap):
                            pmsg = pp_msg.tile([128, dout], F32, tag="pmsg")
                            if node_in + 4 <= 128:
                                nc.tensor.matmul(
                                    pmsg[:], gbuf[0:node_in + 4, col:col + 128],
                                    mwp[0][:], start=True, stop=True)
                            else:
                                nc.tensor.matmul(
                                    pmsg[:], gbuf[0:128, col:col + 128],
                                    mwp[0][:], start=True, stop=False)
                                nc.tensor.matmul(
                                    pmsg[:], eabuf[:, k * 128:k * 128 + 128],
                                    mwp[1][:], start=False, stop=True)
                            sc_ap = ivdb[:, col // 128:col // 128 + 1]
                            if k % 2 == 0:
                                nc.scalar.activation(
                                    out_ap, pmsg[:], AF.Prelu,
                                    scale=sc_ap, alpha=LRELU)
                            else:
                                nc.vector._custom_dve(
                                    _get_lrelu_op(), out=out_ap,
                                    in0=pmsg[:], s0=sc_ap, imm2=LRELU)

                        for ei, ks in enumerate(sched):
                            first = ei == 0
                            last_e = ei == n_em - 1
                            if len(ks) == 2:
                                m2 = p_msg.tile([128, 2 * dout], F8, tag="msg")
                                for j, k in enumerate(ks):
                                    _msg_one(k, wcols[k],
                                             m2[:, j * dout:(j + 1) * dout])
                                lhs3 = m2[:].rearrange(
                                    "p (j d) -> p j d", d=dout)
                                c_sel = wcols[ks[0]]
                                rhs3 = selbuf[:, c_sel:c_sel + 256].rearrange(
                                    "p (j n) -> p j n", n=128)
                                for ch in range(nch):
                                    c0, c1 = ch * 128, min(dout, ch * 128 + 128)
                                    nc.tensor.matmul(
                                        pagg[0:c1 - c0,
                                             ch * 128:ch * 128 + 128],
                                        lhs3[:, :, c0:c1],
                                        rhs3,
                                        start=(first and ch == 0),
                                        stop=(last_e and ch == nch - 1),
                                        perf_mode=mybir.MatmulPerfMode.DoubleRow,
                                        skip_group_check=True)
                            else:
                                k = ks[0]
                                m1 = p_msg.tile([128, dout], msg_dt,
                                                tag="msg1")
                                _msg_one(k, wcols[k], m1[:])
                                for ch in range(nch):
                                    c0, c1 = ch * 128, min(dout, ch * 128 + 128)
                                    nc.tensor.matmul(
                                        pagg[0:c1 - c0,
                                             ch * 128:ch * 128 + 128],
                                        m1[:, c0:c1],
                                        selbuf[:, wcols[k]:wcols[k] + 128],
                                        start=(first and ch == 0),
                                        stop=(last_e and ch == nch - 1),
                                        skip_group_check=True)

                        # ----- window update
                        lhs_list = []
                        for ch in range(nch):
                            c0, c1 = ch * 128, min(dout, ch * 128 + 128)
                            a = p_aggs.tile([128, 128], F16, tag="aggs")
                            nc.vector.tensor_copy(
                                a[0:c1 - c0, :],
                                pagg[0:c1 - c0, ch * 128:ch * 128 + 128])
                            lhs_list.append(a[0:c1 - c0, :])
                        for r in range(0, node_in, 128):
                            r1 = min(node_in, r + 128)
                            lhs_list.append(
                                hlocT[r:r1, w * WIN:w * WIN + 128])
                        uws = W[f"uw{li}"]
                        assert len(uws) == len(lhs_list)
                        pupd = pp_upd.tile([128, max(HID)], F32, tag="pupd")
                        for i, lhs in enumerate(lhs_list):
                            nc.tensor.matmul(
                                pupd[:, 0:dout], lhs, uws[i][:],
                                start=(i == 0), stop=False)
                        nc.tensor.matmul(
                            pupd[:, 0:dout], ones_row[:], W[f"ub{li}"][:],
                            start=False, stop=True)
                        if not last:
                            hn_ap = hna[:, w * 128:w * 128 + dout]
                        else:
                            hn = p_hn.tile([128, max(HID)], F16, tag="hnext")
                            hn_ap = hn[:, 0:dout]
                        nc.scalar.activation(
                            hn_ap, pupd[:, 0:dout], AF.Prelu, alpha=LRELU)
                        if not last:
                            for ch in range(nch):
                                c0, c1 = ch * 128, min(dout, ch * 128 + 128)
                                pt = pp_upd.tile([128, 128], F16, tag="pupd")
                                nc.tensor.transpose(
                                    pt[0:c1 - c0, :],
                                    hna[:, w * 128 + c0:w * 128 + c1],
                                    ident_sb[:])
                                nc.vector.tensor_copy(
                                    hnextT[c0:c1, w * WIN:w * WIN + 128],
                                    pt[0:c1 - c0, :])
                            # stage finished windows into the AllGather
                            # input (node-major rows) and fire the chunked
                            # AllGather into the table
                            hd = dout
                            for ci, cws in enumerate(chunks):
                                if w != cws[-1]:
                                    continue
                                r0 = cws[0] * 128
                                r1 = (cws[-1] + 1) * 128
                                g0 = chunk_gbase[ci]
                                g1 = g0 + n_cores * chunk_rows[ci]
                                # cast f16 -> fp8 during SWDGE staging
                                nc.gpsimd.dma_start(
                                    ag_in[li][r0:r1, :].rearrange(
                                        "(w p) d -> p w d", p=128),
                                    hna[:, r0:r1].rearrange(
                                        "p (w d) -> p w d", d=128)
                                    [:, :, 0:hd])
                                nc.gpsimd.collective_compute(
                                    "AllGather",
                                    mybir.AluOpType.bypass,
                                    replica_groups=[core_ids],
                                    ins=[ag_in[li][r0:r1, :].opt()],
                                    outs=[ag_out[li][g0:g1, :].opt()],
                                )
                                # expand fp8 chunk into 256B-row f16 table
                                nc.gpsimd.dma_start(
                                    H[li][g0:g1, 0:hd],
                                    ag_out[li][g0:g1, :])
                        else:
                            nc.tensor.matmul(
                                psum_pool[:, 0:dout],
                                selB_sb[:, w * NG:(w + 1) * NG],
                                hn_ap,
                                start=(w == 0), stop=False,
                                skip_group_check=True)
                            nc.tensor.matmul(
                                psum_pool[:, dout:dout + 1],
                                selB_sb[:, w * NG:(w + 1) * NG],
                                ones_col[:],
                                start=False, stop=(w == n_win - 1),
                                skip_group_check=True)

                # ----- end of layer
                if not last:
                    gixall = p_gix.tile([128, e_pad // 16], I16, tag="gixall")
                    nc.sync.dma_start(gixall[:], D["gidx"][:])
                    hlocT = hnextT
                node_in = dout

            # ========================= pooling finale + MLP (replicated)
            gp_sb = p_gath.tile([NG, FP], F32, tag="gath", name="gp_sb")
            nc.vector.tensor_copy(gp_sb[:], psum_pool[:])
            nc.sync.dma_start(gp_in[:], gp_sb[:])
            nc.gpsimd.collective_compute(
                "AllGather",
                mybir.AluOpType.bypass,
                replica_groups=[core_ids],
                ins=[gp_in.opt()],
                outs=[gp_out.opt()],
            )
            gall = p_gath.tile([NG, FP * cfg.n_cores], F32, tag="gath",
                               name="gall")
            nc.sync.dma_start(
                gall[:].rearrange("g (f r) -> g f r", r=cfg.n_cores),
                gp_out[:].rearrange("(r g) f -> g f r", g=NG))
            gsum = p_gath.tile([NG, FP], F32, tag="gath", name="gsum")
            nc.vector.tensor_reduce(
                gsum[:],
                gall[:].rearrange("g (f r) -> g f r", r=cfg.n_cores),
                axis=mybir.AxisListType.X,
                op=mybir.AluOpType.add)
            rec = p_small.tile([NG, 1], F32, tag="rec")
            nc.vector.reciprocal(rec[:], gsum[:, HID[2]:HID[2] + 1])
            g16 = p_small.tile([NG, HID[2]], F16, tag="g16")
            nc.scalar.activation(g16[:], gsum[:, 0:HID[2]], AF.Identity,
                                 scale=rec[:])
            gT = p_small.tile([128, 2 * NG], F16, tag="gT")
            for ch in range(2):
                pt = pp_upd.tile([128, 128], F16, tag="pupd")
                nc.tensor.transpose(
                    pt[0:128, 0:NG], g16[:, ch * 128:(ch + 1) * 128],
                    ident_sb[0:NG, 0:NG])
                nc.vector.tensor_copy(
                    gT[:, ch * NG:(ch + 1) * NG], pt[0:128, 0:NG])
            pf1 = pp_upd.tile([128, NG], F32, tag="pupd")
            for ch in range(2):
                nc.tensor.matmul(
                    pf1[0:MLP_DIMS[1], :], W["fw0"][ch][:],
                    gT[:, ch * NG:(ch + 1) * NG],
                    start=(ch == 0), stop=(ch == 1))
            f1 = p_small.tile([MLP_DIMS[1], NG], F16, tag="f1")
            nc.scalar.activation(f1[:], pf1[0:MLP_DIMS[1], :], AF.Prelu,
                                 bias=W["fb0"][:], alpha=LRELU)
            pf2 = pp_upd.tile([128, NG], F32, tag="pupd")
            nc.tensor.matmul(pf2[0:MLP_DIMS[2], :], W["fw1"][0][:], f1[:],
                             start=True, stop=True)
            f2 = p_small.tile([MLP_DIMS[2], NG], F16, tag="f2")
            nc.scalar.activation(f2[:], pf2[0:MLP_DIMS[2], :], AF.Prelu,
                                 bias=W["fb1"][:], alpha=LRELU)
            pf3 = pp_upd.tile([128, NG], F32, tag="pupd")
            nc.tensor.matmul(pf3[0:1, :], W["fw2"][0][:], f2[:],
                             start=True, stop=True)
            fout = p_small.tile([1, NG], F32, tag="fo")
            nc.scalar.activation(fout[:], pf3[0:1, :], AF.Identity,
                                 bias=W["fb2"][:])
            nc.sync.dma_start(out_t[:].rearrange("g o -> o g"), fout[:])

    return nc


def kernel(**inputs):
    cfg, in_maps = host_prep(inputs)
    nc = build_program(cfg)
    nc.compile()
    res = run_bass_kernel_spmd(nc, in_maps, core_ids=list(range(cfg.n_cores)))
    return np.asarray(res.results[0]["out"], np.float32)


# revision 23
# speedup vs baseline: 1.0374x; 1.0374x over previous
"""GNN message-passing discriminator on 8 trn2 NeuronCores.

Strategy (edge-parallel by *destination* node, DRAM-resident node features):
  - Nodes sharded npc/core; each edge lives on the core owning its dst.
  - Per-layer node features live in a DRAM table of 256B rows in chunk-major
    slot order (chunk of CHUNK_W windows, then rank, then node). As each
    7-window chunk of updated nodes finishes, the core casts it to fp8
    (DVE copy, off the Pool sequencer), stages it node-major with an SP DMA,
    and AllGathers the chunk -- so the collectives pipeline behind compute
    instead of serializing at the layer boundary. At end of layer, cheap
    SWDGE cast-DMAs expand the fp8 chunks into the f16 256B-row table, and
    the next layer gathers src features straight from HBM with a transposed
    dma_gather. No SBUF feature table, no full-size AllGather.
  - Message MLP: TensorE matmul per 128-edge tile; LeakyReLU and the mean's
    1/deg(dst) scale fuse into one ScalarE/DVE activation per tile. Layers
    0/1 emit messages in fp8 and aggregate stream-adjacent tile pairs with
    one DoubleRow matmul (256-edge contraction); layer 2 keeps f16 messages
    for accuracy.
  - Aggregation: per-tile one-hot fp8 selector matrices (host-built,
    streamed from HBM) matmul'd against messages, accumulating
    feature-major per-128-node-window sums in PSUM.
  - Update MLP per window writes node-major into the hn staging buffer that
    feeds the chunked AllGathers.
  - Global mean-pool via per-window batch-selector matmuls; final MLP runs
    replicated on every core.

Host-side work is integer index prep (sort/bincount/one-hot selectors) and
layout/dtype staging; all float compute runs on device.
"""

import numpy as np

import concourse.bass as bass
import concourse.bacc as bacc
import concourse.mybir as mybir
import concourse.tile as tile
from concourse.bass_utils import run_bass_kernel_spmd

F32 = mybir.dt.float32
F16 = mybir.dt.float16
F8 = mybir.dt.float8e4
I16 = mybir.dt.int16
I32 = mybir.dt.int32
AF = mybir.ActivationFunctionType
NP_F8 = mybir.dt.np(F8)

N_GRAPHS = 32
HID = [64, 128, 256]
MLP_DIMS = [256, 128, 64, 1]
N_CORES = 8

ELEM = 128      # fp16 feature slots per table row upper half (256 bytes)
ROW = 256       # full table row in fp16 elems (512 bytes, KV-page style)
WIN = 128       # nodes per aggregation window
GROUP_W = 7     # windows per gather group
CHUNK_W = 7     # windows per AllGather-input staging DMA
LRELU = 0.2


def _cdiv(a, b):
    return -(-a // b)


_LRELU_OP = None


def _get_lrelu_op():
    """out = max(s*x, 0.2*s*x) in one DVE pass (s per-partition, 0.2 imm)."""
    global _LRELU_OP
    if _LRELU_OP is not None:
        return _LRELU_OP
    import concourse.dve_ops as dops
    from concourse.dve_spec import Spec, Src0, C0, C2, maxx
    name = "LRELU_SCALE_ANT"
    if name not in dops._SUB_OPCODE_FOR_NAME:
        row = max(dops._SUB_OPCODE_FOR_NAME.values()) + 1
        assert row < 0x20
        dops._SUB_OPCODE_FOR_NAME[name] = row
    spec = Spec(
        body=maxx(Src0 * C0, Src0 * C0 * C2),
        reference=lambda in0, in1, c0, c1, c2: np.maximum(
            in0 * c0, in0 * c0 * c2),
    )
    shas = {}
    for ver in ("v3", "v4"):
        try:
            probe = dops.DveOp(name, spec, subdim=False, uops_sha={})
            probe.compile(ver)
        except ValueError as ex:
            import re
            shas[ver] = re.search(r"\(" + ver + r": ([0-9a-f]+)", str(ex)).group(1)
    op = dops.DveOp(name, spec, subdim=False, uops_sha=shas)
    if not any(o.name == name for o in dops.OPS):
        dops.OPS.append(op)
    dops.CUSTOM_DVE_SPECS[name] = spec
    _LRELU_OP = op
    return op


class Cfg:
    pass


# ============================================================ host index prep
def host_prep(inputs, n_cores=N_CORES):
    x = np.asarray(inputs["x"], np.float32)
    ei = np.asarray(inputs["edge_index"], np.int64)
    ea = np.asarray(inputs["edge_attr"], np.float32)
    batch = np.asarray(inputs["batch"], np.int64)

    n_nodes, node_dim = x.shape
    n_edges = ei.shape[1]

    cfg = Cfg()
    cfg.n_cores = n_cores
    cfg.n_nodes = n_nodes
    cfg.node_dim = node_dim
    cfg.n_graphs = N_GRAPHS
    npc = n_nodes // n_cores
    assert npc * n_cores == n_nodes
    cfg.npc = npc
    cfg.n_win = _cdiv(npc, WIN)
    cfg.last_win_nodes = npc - (cfg.n_win - 1) * WIN
    win_pad = cfg.n_win * WIN
    cfg.win_pad = win_pad
    # chunk-major slot space (chunk k of CHUNK_W windows, then rank, then
    # node) so chunked AllGather outputs are contiguous table ranges; gather
    # idx is int16 so bucket-split at 32768
    chunks = [list(range(g, min(g + CHUNK_W, cfg.n_win)))
              for g in range(0, cfg.n_win, CHUNK_W)]
    cfg.chunks = chunks
    cfg.chunk_rows = [len(ws) * WIN for ws in chunks]
    gb = np.cumsum([0] + [n_cores * cr for cr in cfg.chunk_rows])
    cfg.chunk_gbase = gb[:-1]
    cfg.slots = int(gb[-1])
    assert cfg.slots == n_cores * win_pad
    cfg.buck0 = min(32768, cfg.slots)
    assert cfg.slots - cfg.buck0 < 32768

    src = ei[0].astype(np.int64)
    dst = ei[1].astype(np.int64)
    deg = np.bincount(dst, minlength=n_nodes).astype(np.float32)
    inv_deg = (1.0 / np.maximum(deg, 1.0)).astype(np.float32)

    c_of = np.arange(n_nodes) // npc
    r_of = np.arange(n_nodes) % npc
    w_of = r_of // WIN
    k_of = w_of // CHUNK_W
    cr = np.asarray(cfg.chunk_rows, np.int64)
    gbase = np.asarray(cfg.chunk_gbase, np.int64)
    first_w = np.asarray([ws[0] for ws in chunks], np.int64)
    slot = (gbase[k_of] + c_of * cr[k_of]
            + (r_of - first_w[k_of] * WIN)).astype(np.int64)
    slot_of_node = slot

    sslot = slot[src]
    ecore = dst // npc
    ewin = (dst % npc) // WIN
    ebuck = (sslot >= cfg.buck0).astype(np.int64)

    key = (ecore * cfg.n_win + ewin) * 2 + ebuck
    cnt = np.bincount(key, minlength=n_cores * cfg.n_win * 2).reshape(
        n_cores, cfg.n_win, 2)
    T = np.maximum(_cdiv(cnt.max(axis=0), 128), 1)   # [n_win, 2]
    cfg.T = T
    cfg.n_tiles = int(T.sum())
    cfg.e_pad = cfg.n_tiles * 128

    groups = [list(range(g, min(g + GROUP_W, cfg.n_win)))
              for g in range(0, cfg.n_win, GROUP_W)]
    cfg.groups = groups

    # padded stream order: per group: [A segs of its windows] [B segs]
    seg_off = {}
    pos = 0
    for ws in groups:
        for b in (0, 1):
            for w in ws:
                seg_off[(w, b)] = pos
                pos += int(T[w, b]) * 128
    assert pos == cfg.e_pad
    cfg.seg_off = seg_off

    order = np.lexsort((ebuck, ewin, ecore))
    src_s = sslot[order]
    dst_s = dst[order]
    ea_s = ea[order]
    inv_s = inv_deg[dst[order]]

    ck = (ecore[order] * cfg.n_win + ewin[order]) * 2 + ebuck[order]
    seg_starts = np.searchsorted(ck, np.arange(n_cores * cfg.n_win * 2))
    seg_ends = np.append(seg_starts[1:], n_edges)

    e_pad = cfg.e_pad
    in_maps = []
    wts = _pack_weights(inputs, node_dim)
    ident = np.eye(128, dtype=np.float16)

    for c in range(n_cores):
        g_idx = np.zeros(e_pad, np.int64)
        buck_flag = np.zeros(e_pad, np.bool_)
        e_a = np.zeros((4, e_pad), np.float32)
        invd = np.zeros(e_pad, np.float32)
        selcol = np.full(e_pad, -1, np.int64)

        for w in range(cfg.n_win):
            for b in (0, 1):
                s0 = seg_starts[(c * cfg.n_win + w) * 2 + b]
                s1 = seg_ends[(c * cfg.n_win + w) * 2 + b]
                n = s1 - s0
                o = seg_off[(w, b)]
                assert n <= T[w, b] * 128
                if n:
                    buck_flag[o:o + n] = bool(b)
                    g_idx[o:o + n] = src_s[s0:s1] - (cfg.buck0 if b else 0)
                    e_a[:3, o:o + n] = ea_s[s0:s1].T
                    e_a[3, o:o + n] = 1.0
                    invd[o:o + n] = inv_s[s0:s1]
                    selcol[o:o + n] = (dst_s[s0:s1] % npc) - w * WIN

        gi = np.zeros((128, e_pad // 16), np.int16)
        base = g_idx.astype(np.int16).reshape(-1, 16).T
        for k in range(8):
            gi[16 * k:16 * k + 16] = base

        # layer-0 edge stream: [x[src](10) | ea(3) | 1] fp16, feature-major
        xe = np.zeros((node_dim + 4, e_pad), np.float16)
        edge_valid = selcol >= 0

        sel = np.zeros((128, cfg.n_tiles * 128), np.uint8)
        tt = np.arange(e_pad) // 128
        ee = np.arange(e_pad) % 128
        m = selcol >= 0
        sel[ee[m], tt[m] * 128 + selcol[m]] = 0x38

        xt = np.zeros((node_dim, win_pad), np.float16)
        xt[:, :npc] = x[c * npc:(c + 1) * npc].astype(np.float16).T

        sb = np.zeros((128, cfg.n_win * N_GRAPHS), np.uint8)
        bl = batch[c * npc:(c + 1) * npc].astype(np.int64)
        pp = np.arange(npc) % WIN
        ww = np.arange(npc) // WIN
        sb[pp, ww * N_GRAPHS + bl] = 0x38

        xsrc_slot = np.zeros((cfg.slots, node_dim), np.float16)
        xsrc_slot[slot_of_node] = x.astype(np.float16)
        gsl = g_idx + np.where(buck_flag, cfg.buck0, 0)
        xe[:node_dim, :] = xsrc_slot[gsl].T
        xe[node_dim:node_dim + 4, :] = e_a.astype(np.float16)
        xe[:, ~edge_valid] = 0.0

        m_ = {
            "xeT": xe,
            "gidx": gi,
            "eaT": e_a.astype(np.float16),
            "invd": invd.reshape(-1, 128).T.astype(np.float32).copy(),
            "sel": sel.view(NP_F8),
            "xT_loc": xt,
            "selB": sb.view(NP_F8),
            "ident": ident,
        }
        m_.update(wts)
        in_maps.append(m_)
    return cfg, in_maps


def _pack_weights(inputs, node_dim):
    wts = {}
    node_in = node_dim
    for li in range(len(HID)):
        mw = np.asarray(inputs[f"mw{li}"], np.float32)
        mb = np.asarray(inputs[f"mb{li}"], np.float32)
        wts[f"mwp{li}"] = np.concatenate([mw, mb[None, :]], axis=0)
        wts[f"uw{li}"] = np.asarray(inputs[f"uw{li}"], np.float32)
        wts[f"ub{li}"] = np.asarray(inputs[f"ub{li}"], np.float32)[None, :]
        node_in = HID[li]
    for li in range(len(MLP_DIMS) - 1):
        wts[f"fw{li}"] = np.asarray(inputs[f"fw{li}"], np.float32)
        wts[f"fb{li}"] = np.asarray(
            inputs[f"fb{li}"], np.float32).reshape(-1, 1)
    return wts


# =============================================================== bass builder
def build_program(cfg):
    nc = bacc.Bacc(
        "TRN2",
        target_bir_lowering=False,
        debug=False,
        enable_asserts=False,
        num_devices=cfg.n_cores,
    )
    n_win, n_tiles, e_pad = cfg.n_win, cfg.n_tiles, cfg.e_pad
    win_pad = cfg.win_pad
    NG = cfg.n_graphs
    T = cfg.T
    groups = cfg.groups
    chunks = cfg.chunks
    chunk_rows = cfg.chunk_rows
    chunk_gbase = [int(v) for v in cfg.chunk_gbase]
    core_ids = list(range(cfg.n_cores))
    slots = cfg.slots
    buck0 = cfg.buck0
    n_cores = cfg.n_cores

    D = {}

    def din(name, shape, dt):
        D[name] = nc.dram_tensor(name, list(shape), dt, kind="ExternalInput")

    din("xeT", (cfg.node_dim + 4, e_pad), F16)
    din("gidx", (128, e_pad // 16), I16)
    din("eaT", (4, e_pad), F16)
    din("invd", (128, n_tiles), F32)
    din("sel", (128, n_tiles * 128), F8)
    din("xT_loc", (cfg.node_dim, win_pad), F16)
    din("selB", (128, n_win * NG), F8)
    din("ident", (128, 128), F16)
    node_in = cfg.node_dim
    for li, dout in enumerate(HID):
        din(f"mwp{li}", (node_in + 4, dout), F32)
        din(f"uw{li}", (dout + node_in, dout), F32)
        din(f"ub{li}", (1, dout), F32)
        node_in = dout
    for li in range(len(MLP_DIMS) - 1):
        din(f"fw{li}", (MLP_DIMS[li], MLP_DIMS[li + 1]), F32)
        din(f"fb{li}", (MLP_DIMS[li + 1], 1), F32)
    out_t = nc.dram_tensor("out", [NG, 1], F32, kind="ExternalOutput")

    # group extents in the padded stream
    g_meta = []
    for ws in groups:
        nA = int(sum(T[w, 0] for w in ws)) * 128
        nB = int(sum(T[w, 1] for w in ws)) * 128
        g_meta.append((cfg.seg_off[(ws[0], 0)], nA, nB))
    max_g_cols = max(nA + nB for _, nA, nB in g_meta)
    max_w_cols = int((T[:, 0] + T[:, 1]).max()) * 128

    from contextlib import ExitStack
    with ExitStack() as _es:
        tc = _es.enter_context(tile.TileContext(nc))
        p_res = _es.enter_context(tc.tile_pool(name="res", bufs=1))
        p_wts = _es.enter_context(tc.tile_pool(name="wts", bufs=1))
        p_gath = _es.enter_context(tc.tile_pool(name="gath", bufs=2))
        p_sel = _es.enter_context(tc.tile_pool(name="selp", bufs=2))
        p_gix = _es.enter_context(tc.tile_pool(name="gix", bufs=2))
        p_ivd = _es.enter_context(tc.tile_pool(name="ivd", bufs=2))
        p_ea = _es.enter_context(tc.tile_pool(name="eal2", bufs=2))
        p_msg = _es.enter_context(tc.tile_pool(name="msg", bufs=6))
        p_aggs = _es.enter_context(tc.tile_pool(name="aggs", bufs=4))
        p_hloc = _es.enter_context(tc.tile_pool(name="hloc", bufs=2))
        p_hna = _es.enter_context(tc.tile_pool(name="hna", bufs=2))
        p_hn = _es.enter_context(tc.tile_pool(name="hnext", bufs=3))
        p_small = _es.enter_context(tc.tile_pool(name="small", bufs=1))
        pp_msg = _es.enter_context(tc.tile_pool(name="pmsg", bufs=4, space="PSUM"))
        pp_agg = _es.enter_context(tc.tile_pool(name="pagg", bufs=2, space="PSUM"))
        pp_upd = _es.enter_context(tc.tile_pool(name="pupd", bufs=1, space="PSUM"))
        pp_pool = _es.enter_context(tc.tile_pool(name="ppool", bufs=1, space="PSUM"))
        p_dram = _es.enter_context(tc.tile_pool(name="dram", bufs=1, space="DRAM"))
        if True:
            selB_sb = p_res.tile([128, n_win * NG], F8, tag="selB")
            ident_sb = p_res.tile([128, 128], F16, tag="ident")
            ones_row = p_res.tile([1, 128], F16, tag="ones_r")
            ones_col = p_res.tile([128, 1], F16, tag="ones_c")

            nc.sync.dma_start(selB_sb[:], D["selB"][:])
            nc.sync.dma_start(ident_sb[:], D["ident"][:])
            nc.vector.memset(ones_row[:], 1.0)
            nc.vector.memset(ones_col[:], 1.0)

            # weights -> SBUF fp16 (cast during SWDGE DMA)
            W = {}
            node_in = cfg.node_dim
            for li, dout in enumerate(HID):
                mw_chunks = []
                for k, r in enumerate(range(0, node_in + 4, 128)):
                    r1 = min(r + 128, node_in + 4)
                    t = p_wts.tile([r1 - r, dout], F16, tag=f"mwp{li}_{k}")
                    nc.gpsimd.dma_start(t[:], D[f"mwp{li}"][r:r1, :])
                    mw_chunks.append(t)
                W[f"mwp{li}"] = mw_chunks
                chunks_w = []
                for r in list(range(0, dout, 128)):
                    chunks_w.append((r, min(r + 128, dout)))
                for r in list(range(0, node_in, 128)):
                    chunks_w.append((dout + r, dout + min(r + 128, node_in)))
                uws = []
                for k, (r0, r1) in enumerate(chunks_w):
                    t = p_wts.tile([r1 - r0, dout], F16, tag=f"uw{li}_{k}")
                    nc.gpsimd.dma_start(t[:], D[f"uw{li}"][r0:r1, :])
                    uws.append(t)
                W[f"uw{li}"] = uws
                t = p_wts.tile([1, dout], F16, tag=f"ub{li}")
                nc.gpsimd.dma_start(t[:], D[f"ub{li}"][:])
                W[f"ub{li}"] = t
                node_in = dout
            for li in range(len(MLP_DIMS) - 1):
                fws = []
                for k, r in enumerate(range(0, MLP_DIMS[li], 128)):
                    r1 = min(r + 128, MLP_DIMS[li])
                    t = p_wts.tile([r1 - r, MLP_DIMS[li + 1]], F16,
                                   tag=f"fw{li}_{k}")
                    nc.gpsimd.dma_start(t[:], D[f"fw{li}"][r:r1, :])
                    fws.append(t)
                W[f"fw{li}"] = fws
                t = p_wts.tile([MLP_DIMS[li + 1], 1], F32, tag=f"fb{li}")
                nc.sync.dma_start(t[:], D[f"fb{li}"][:])
                W[f"fb{li}"] = t

            hlocT = p_hloc.tile([cfg.node_dim, win_pad], F16, tag="hloc")
            nc.sync.dma_start(hlocT[:], D["xT_loc"][:])

            # per-layer DRAM feature tables (256B rows for dma_gather).
            # L0's h1 (64 feats) AllGathers tight in fp8, then a local
            # cast-DMA expands each chunk into H0; L1's h2 (128 feats)
            # AllGathers straight into H1.
            ag_in = {}
            ag_out = {}
            H = {}
            for li, hd in ((0, 64), (1, 128)):
                ag_in[li] = p_dram.tile([win_pad, hd], F8, tag=f"agi{li}",
                                        name=f"agi{li}")
                ag_out[li] = p_dram.tile([slots, hd], F8, tag=f"ago{li}",
                                         name=f"ago{li}")
                H[li] = p_dram.tile([slots, ELEM], F16, tag=f"H{li}",
                                    name=f"H{li}")

            FP = HID[2] + 1
            gp_in = p_dram.tile([NG, FP], F32, tag="gpi")
            gp_out = p_dram.tile([cfg.n_cores * NG, FP], F32, tag="gpo",
                                 addr_space="Shared")
            psum_pool = pp_pool.tile([NG, FP], F32)

            # ========================= layers
            node_in = cfg.node_dim
            gixall = None
            for li, dout in enumerate(HID):
                mwp = W[f"mwp{li}"]
                nch = _cdiv(dout, 128)
                last = li == len(HID) - 1
                hnextT = None
                hna = None
                if not last:
                    hnextT = p_hloc.tile([dout, win_pad], F16, tag="hloc")
                    hna = p_hna.tile([128, win_pad], F16, tag="hna")
                    if dout < 128:
                        # pad feature columns feed H's v-rows (gathered but
                        # never consumed); zero them so sims see no uninit
                        nc.vector.memset(hna[:], 0.0)

                for gi_, ws in enumerate(groups):
                    start, nA, nB = g_meta[gi_]
                    ncols = nA + nB
                    selbuf = p_sel.tile([128, max_g_cols], F8, tag="selp")
                    ivdb = p_ivd.tile([128, max_g_cols // 128], F32, tag="ivd")
                    nc.sync.dma_start(
                        ivdb[:, 0:ncols // 128],
                        D["invd"][:, start // 128:(start + ncols) // 128])
                    nc.sync.dma_start(
                        selbuf[:, 0:ncols],
                        D["sel"][:, start:start + ncols])
                    if li == 0:
                        gbuf = p_gath.tile([cfg.node_dim + 4, max_g_cols],
                                           F16, tag="gath", name="gbuf")
                        nc.sync.dma_start(
                            gbuf[:, 0:ncols],
                            D["xeT"][:, start:start + ncols])
                    else:
                        gbuf = p_gath.tile([128, max_g_cols], F16, tag="gath",
                                           name="gbuf")
                        Hsrc = H[li - 1]
                        for b, coff, n_b in ((0, 0, nA), (1, nA, nB)):
                            if n_b == 0:
                                continue
                            r0 = 0 if b == 0 else buck0
                            r1 = buck0 if b == 0 else slots
                            if r1 <= r0:
                                # bucket empty (all-pad tiles): any valid rows
                                r0, r1 = 0, slots
                            nc.gpsimd.dma_gather(
                                gbuf[:, coff:coff + n_b].rearrange(
                                    "p (o n) -> p o n", o=1),
                                Hsrc[r0:r1, :],
                                gixall[:, (start + coff) // 16:
                                       (start + coff + n_b) // 16],
                                num_idxs=n_b,
                                num_idxs_reg=n_b,
                                elem_size=ELEM,
                                transpose=True,
                                single_packet=False,
                            )
                        if node_in + 4 <= 128:
                            nc.sync.dma_start(
                                gbuf[node_in:node_in + 4, 0:ncols],
                                D["eaT"][:, start:start + ncols])

                    # window-major processing; stream offsets are bucket-major
                    aoff = 0
                    boff = nA
                    for w in ws:
                        t0, t1 = int(T[w, 0]), int(T[w, 1])
                        wcols = []
                        for k in range(t0):
                            wcols.append(aoff + k * 128)
                        for k in range(t1):
                            wcols.append(boff + k * 128)
                        aoff += t0 * 128
                        boff += t1 * 128

                        eabuf = None
                        if node_in + 4 > 128:
                            eabuf = p_ea.tile([4, max_w_cols], F16, tag="eal2")
                            nc.sync.dma_start(
                                eabuf[:, 0:t0 * 128],
                                D["eaT"][:, start + wcols[0]:
                                         start + wcols[0] + t0 * 128])
                            nc.sync.dma_start(
                                eabuf[:, t0 * 128:(t0 + t1) * 128],
                                D["eaT"][:, start + wcols[t0]:
                                         start + wcols[t0] + t1 * 128])

                        pagg = pp_agg.tile([128, nch * 128], F32, tag="pagg")
                        # L0/L1: messages come out in fp8 and stream-adjacent
                        # tile pairs aggregate with one DoubleRow matmul
                        # (256-edge contraction). L2 keeps fp16 messages for
                        # accuracy (it feeds h3 / the output most directly).
                        pair_fp8 = li < 2
                        msg_dt = F8 if pair_fp8 else F16
                        sched = []
                        for rs, re_ in ((0, t0), (t0, t0 + t1)):
                            k = rs
                            while pair_fp8 and k + 1 < re_:
                                sched.append((k, k + 1))
                                k += 2
                            while k < re_:
                                sched.append((k,))
                                k += 1
                        n_em = len(sched)

                        def _msg_one(k, col, out_ap):
                            pmsg = pp_msg.tile([128, dout], F32, tag="pmsg")
                            if node_in + 4 <= 128:
                                nc.tensor.matmul(
                                    pmsg[:], gbuf[0:node_in + 4, col:col + 128],
                                    mwp[0][:], start=True, stop=True)
                            else:
                                nc.tensor.matmul(
                                    pmsg[:], gbuf[0:128, col:col + 128],
                                    mwp[0][:], start=True, stop=False)
                                nc.tensor.matmul(
                                    pmsg[:], eabuf[:, k * 128:k * 128 + 128],
                                    mwp[1][:], start=False, stop=True)
                            sc_ap = ivdb[:, col // 128:col // 128 + 1]
                            if k % 2 == 0:
                                nc.scalar.activation(
                                    out_ap, pmsg[:], AF.Prelu,
                                    scale=sc_ap, alpha=LRELU)
                            else:
                                nc.vector._custom_dve(
                                    _get_lrelu_op(), out=out_ap,
                                    in0=pmsg[:], s0=sc_ap, imm2=LRELU)

                        for ei, ks in enumerate(sched):
                            first = ei == 0
                            last_e = ei == n_em - 1
                            if len(ks) == 2:
                                m2 = p_msg.tile([128, 2 * dout], F8, tag="msg")
                                for j, k in enumerate(ks):
                                    _msg_one(k, wcols[k],
                                             m2[:, j * dout:(j + 1) * dout])
                                lhs3 = m2[:].rearrange(
                                    "p (j d) -> p j d", d=dout)
                                c_sel = wcols[ks[0]]
                                rhs3 = selbuf[:, c_sel:c_sel + 256].rearrange(
                                    "p (j n) -> p j n", n=128)
                                for ch in range(nch):
                                    c0, c1 = ch * 128, min(dout, ch * 128 + 128)
                                    nc.tensor.matmul(
                                        pagg[0:c1 - c0,
                                             ch * 128:ch * 128 + 128],
                                        lhs3[:, :, c0:c1],
                                        rhs3,
                                        start=(first and ch == 0),
                                        stop=(last_e and ch == nch - 1),
                                        perf_mode=mybir.MatmulPerfMode.DoubleRow,
                                        skip_group_check=True)
                            else:
                                k = ks[0]
                                m1 = p_msg.tile([128, dout], msg_dt,
                                                tag="msg1")
                                _msg_one(k, wcols[k], m1[:])
                                for ch in range(nch):
                                    c0, c1 = ch * 128, min(dout, ch * 128 + 128)
                                    nc.tensor.matmul(
                                        pagg[0:c1 - c0,
                                             ch * 128:ch * 128 + 128],
                                        m1[:, c0:c1],
                                        selbuf[:, wcols[k]:wcols[k] + 128],
                                        start=(first and ch == 0),
                                        stop=(last_e and ch == nch - 1),
                                        skip_group_check=True)

                        # ----- window update
                        lhs_list = []
                        for ch in range(nch):
                            c0, c1 = ch * 128, min(dout, ch * 128 + 128)
                            a = p_aggs.tile([128, 128], F16, tag="aggs")
                            nc.vector.tensor_copy(
                                a[0:c1 - c0, :],
                                pagg[0:c1 - c0, ch * 128:ch * 128 + 128])
                            lhs_list.append(a[0:c1 - c0, :])
                        for r in range(0, node_in, 128):
                            r1 = min(node_in, r + 128)
                            lhs_list.append(
                                hlocT[r:r1, w * WIN:w * WIN + 128])
                        uws = W[f"uw{li}"]
                        assert len(uws) == len(lhs_list)
                        pupd = pp_upd.tile([128, max(HID)], F32, tag="pupd")
                        for i, lhs in enumerate(lhs_list):
                            nc.tensor.matmul(
                                pupd[:, 0:dout], lhs, uws[i][:],
                                start=(i == 0), stop=False)
                        nc.tensor.matmul(
                            pupd[:, 0:dout], ones_row[:], W[f"ub{li}"][:],
                            start=False, stop=True)
                        if not last:
                            hn_ap = hna[:, w * 128:w * 128 + dout]
                        else:
                            hn = p_hn.tile([128, max(HID)], F16, tag="hnext")
                            hn_ap = hn[:, 0:dout]
                        nc.scalar.activation(
                            hn_ap, pupd[:, 0:dout], AF.Prelu, alpha=LRELU)
                        if not last:
                            for ch in range(nch):
                                c0, c1 = ch * 128, min(dout, ch * 128 + 128)
                                pt = pp_upd.tile([128, 128], F16, tag="pupd")
                                nc.tensor.transpose(
                                    pt[0:c1 - c0, :],
                                    hna[:, w * 128 + c0:w * 128 + c1],
                                    ident_sb[:])
                                nc.vector.tensor_copy(
                                    hnextT[c0:c1, w * WIN:w * WIN + 128],
                                    pt[0:c1 - c0, :])
                            # stage finished windows into the AllGather
                            # input (node-major rows) and fire the chunked
                            # AllGather into the table
                            hd = dout
                            for ci, cws in enumerate(chunks):
                                if w != cws[-1]:
                                    continue
                                r0 = cws[0] * 128
                                r1 = (cws[-1] + 1) * 128
                                g0 = chunk_gbase[ci]
                                g1 = g0 + n_cores * chunk_rows[ci]
                                # cast f16 -> fp8 during SWDGE staging
                                nc.gpsimd.dma_start(
                                    ag_in[li][r0:r1, :].rearrange(
                                        "(w p) d -> p w d", p=128),
                                    hna[:, r0:r1].rearrange(
                                        "p (w d) -> p w d", d=128)
                                    [:, :, 0:hd])
                                nc.gpsimd.collective_compute(
                                    "AllGather",
                                    mybir.AluOpType.bypass,
                                    replica_groups=[core_ids],
                                    ins=[ag_in[li][r0:r1, :].opt()],
                                    outs=[ag_out[li][g0:g1, :].opt()],
                                )
                                # expand fp8 chunk into 256B-row f16 table
                                nc.gpsimd.dma_start(
                                    H[li][g0:g1, 0:hd],
                                    ag_out[li][g0:g1, :])
                        else:
                            nc.tensor.matmul(
                                psum_pool[:, 0:dout],
                                selB_sb[:, w * NG:(w + 1) * NG],
                                hn_ap,
                                start=(w == 0), stop=False,
                                skip_group_check=True)
                            nc.tensor.matmul(
                                psum_pool[:, dout:dout + 1],
                                selB_sb[:, w * NG:(w + 1) * NG],
                                ones_col[:],
                                start=False, stop=(w == n_win - 1),
                                skip_group_check=True)

                # ----- end of layer
                if not last:
                    gixall = p_gix.tile([128, e_pad // 16], I16, tag="gixall")
                    nc.sync.dma_start(gixall[:], D["gidx"][:])
                    hlocT = hnextT
                node_in = dout

            # ========================= pooling finale + MLP (replicated)
            gp_sb = p_gath.tile([NG, FP], F32, tag="gath", name="gp_sb")
            nc.vector.tensor_copy(gp_sb[:], psum_pool[:])
            nc.sync.dma_start(gp_in[:], gp_sb[:])
            nc.gpsimd.collective_compute(
                "AllGather",
                mybir.AluOpType.bypass,
                replica_groups=[core_ids],
                ins=[gp_in.opt()],
                outs=[gp_out.opt()],
            )
            gall = p_gath.tile([NG, FP * cfg.n_cores], F32, tag="gath",
                               name="gall")
            nc.sync.dma_start(
                gall[:].rearrange("g (f r) -> g f r", r=cfg.n_cores),
                gp_out[:].rearrange("(r g) f -> g f r", g=NG))
            gsum = p_gath.tile([NG, FP], F32, tag="gath", name="gsum")
            nc.vector.tensor_reduce(
                gsum[:],
                gall[:].rearrange("g (f r) -> g f r", r=cfg.n_cores),
                axis=mybir.AxisListType.X,
                op=mybir.AluOpType.add)
            rec = p_small.tile([NG, 1], F32, tag="rec")
            nc.vector.reciprocal(rec[:], gsum[:, HID[2]:HID[2] + 1])
            g16 = p_small.tile([NG, HID[2]], F16, tag="g16")
            nc.scalar.activation(g16[:], gsum[:, 0:HID[2]], AF.Identity,
                                 scale=rec[:])
            gT = p_small.tile([128, 2 * NG], F16, tag="gT")
            for ch in range(2):
                pt = pp_upd.tile([128, 128], F16, tag="pupd")
                nc.tensor.transpose(
                    pt[0:128, 0:NG], g16[:, ch * 128:(ch + 1) * 128],
                    ident_sb[0:NG, 0:NG])
                nc.vector.tensor_copy(
                    gT[:, ch * NG:(ch + 1) * NG], pt[0:128, 0:NG])
            pf1 = pp_upd.tile([128, NG], F32, tag="pupd")
            for ch in range(2):
                nc.tensor.matmul(
                    pf1[0:MLP_DIMS[1], :], W["fw0"][ch][:],
                    gT[:, ch * NG:(ch + 1) * NG],
                    start=(ch == 0), stop=(ch == 1))
            f1 = p_small.tile([MLP_DIMS[1], NG], F16, tag="f1")
            nc.scalar.activation(f1[:], pf1[0:MLP_DIMS[1], :], AF.Prelu,
                                 bias=W["fb0"][:], alpha=LRELU)
            pf2 = pp_upd.tile([128, NG], F32, tag="pupd")
            nc.tensor.matmul(pf2[0:MLP_DIMS[2], :], W["fw1"][0][:], f1[:],
                             start=True, stop=True)
            f2 = p_small.tile([MLP_DIMS[2], NG], F16, tag="f2")
            nc.scalar.activation(f2[:], pf2[0:MLP_DIMS[2], :], AF.Prelu,
                                 bias=W["fb1"][:], alpha=LRELU)
            pf3 = pp_upd.tile([128, NG], F32, tag="pupd")
            nc.tensor.matmul(pf3[0:1, :], W["fw2"][0][:], f2[:],
                             start=True, stop=True)
            fout = p_small.tile([1, NG], F32, tag="fo")
            nc.scalar.activation(fout[:], pf3[0:1, :], AF.Identity,
                                 bias=W["fb2"][:])
            nc.sync.dma_start(out_t[:].rearrange("g o -> o g"), fout[:])

    return nc


def kernel(**inputs):
    cfg, in_maps = host_prep(inputs)
    nc = build_program(cfg)
    nc.compile()
    res = run_bass_kernel_spmd(nc, in_maps, core_ids=list(range(cfg.n_cores)))
    return np.asarray(res.results[0]["out"], np.float32)
